# revision 41
# baseline (speedup 1.0000x reference)
"""Causal self-attention (T=2048, C=1024, H=16) on 8 trn2 NeuronCores.

Tensor-parallel over heads: core i computes heads 2i, 2i+1 (q/k/v rows
128i:128i+128 of each 1024-row block of wqkv_w, proj_w columns
128i:128i+128), producing a partial output projection; partials are summed
on the host (the all-reduce of the sharding hint).

Per-core Bass/Tile kernel, bf16 matmuls with fp32 PSUM accumulation. The
PE instruction stream is kept dense (HAM re-throttles the PE clock to
1.2GHz after any ~3.4us idle/transpose-only window, and only re-warms
after ~3.4us of continuous matmul activity):
  B. qkvT[j, t] = wqkv.T @ xT, contraction-tile outer so matmuls chase the
     x DMAs; evacuation adds the (per-partition) bias and casts to bf16;
     q rows pre-scaled by 1/sqrt(C) on the host. v's PE transposes into
     v_aug[k, 65] (ones column = softmax denominator via the PV matmul)
     are interleaved with the v matmuls so they never form a
     transpose-only PE window.
  D. per 512-col t-chunk, heads interleaved, PV pipelined PIPE k-tile
     steps behind the scores, and the previous chunk's normalize/proj
     matmuls spread through the j-loop as PE filler while ScalarE
     (exp) catches up:
       sT[k, t] = kT.T @ qT -> exp (bf16, no max-subtraction needed:
       |scores| < ~1 by construction) -> causal affine_select on gpsimd
       (diagonal k-tiles only; fully-masked tiles skipped) ->
       attnT[d, t] += v_aug.T @ exp_w.
     Normalize: 1/sums via Ln+Exp(-x) on ScalarE (one activation table
     set holds both), partition-broadcast via K=1 float32r matmul with
     ones, one DVE multiply.
  E. partialT[o, t] = projT.T @ attn (two K=64 matmuls, one per head).
"""

import sys

if "/opt/trn_rl_repo" not in sys.path:
    sys.path.insert(0, "/opt/trn_rl_repo")

import ml_dtypes
import numpy as np

T = 2048
C = 1024
CH = 512  # t-chunk width (one PSUM bank of fp32)
NT = T // CH  # 4 t-chunks
NK = T // 128  # 16 k-tiles
NCT = C // 128  # 8 contraction tiles
N_CORES = 8
PIPE = 3  # scores->PV pipeline depth in k-tile steps

_CACHE = {}


def _patch_act_tables(bacc_mod, mybir):
    """Make Exp and Ln resolve to the one table set containing both, so the
    kernel needs a single ACT_TABLE_LOAD instead of thrashing between
    exp_and_others and natural_log_exp_and_others (~1.3us per reload)."""
    if getattr(bacc_mod, "_attn_act_patch", False):
        return
    orig = bacc_mod.get_activation_tables
    both = {mybir.ActivationFunctionType.Exp, mybir.ActivationFunctionType.Ln}

    def patched(arch):
        tabs = dict(orig(arch))
        return {
            name: (funcs if name == "natural_log_exp_and_others" else funcs - both)
            for name, funcs in tabs.items()
        }

    bacc_mod.get_activation_tables = patched
    bacc_mod._attn_act_patch = True


def _build():
    import concourse.tile as tile
    from concourse import bacc, mybir

    _patch_act_tables(bacc, mybir)

    F32 = mybir.dt.float32
    F32R = mybir.dt.float32r
    BF16 = mybir.dt.bfloat16
    EXP = mybir.ActivationFunctionType.Exp
    LN = mybir.ActivationFunctionType.Ln
    IS_GE = mybir.AluOpType.is_ge

    nc = bacc.Bacc(
        "TRN2",
        target_bir_lowering=False,
        debug=False,
        enable_asserts=False,
        num_devices=N_CORES,
        num_swdge_queues=4,
    )
    xT = nc.dram_tensor("xT", [C, T], BF16, kind="ExternalInput").ap()
    wqkv = nc.dram_tensor("wqkv", [C, 384], BF16, kind="ExternalInput").ap()
    projT = nc.dram_tensor("projT", [128, C], BF16, kind="ExternalInput").ap()
    identb = nc.dram_tensor("identb", [128, 64], BF16, kind="ExternalInput").ap()
    ones_f = nc.dram_tensor("ones_f", [128, 128], F32R, kind="ExternalInput").ap()
    bias = nc.dram_tensor("bias", [128, 3], F32, kind="ExternalInput").ap()
    # output as contiguous [chunk, o-tile, 128, 512] bf16 tiles: each store is
    # one fully-contiguous 128KB DMA (strided stores run at ~1/3 the queue
    # rate, and bf16 halves both the store bytes and the evacuation time)
    out = nc.dram_tensor("out", [NT, 8, 128, CH], BF16, kind="ExternalOutput").ap()

    with tile.TileContext(nc) as tc:
        with (
            tc.tile_pool(name="big", bufs=1) as big,
            tc.tile_pool(name="expw", bufs=8) as expw_pool,
            tc.tile_pool(name="attn_tmp", bufs=2) as attn_tmp_pool,
            tc.tile_pool(name="outev", bufs=3) as outev_pool,
            tc.tile_pool(name="ps", bufs=1, space="PSUM") as ps,
        ):
            # ---- resident SBUF tensors -------------------------------------
            x_sb = big.tile([128, NCT, T], BF16, name="x_sb")
            w_sb = big.tile([128, NCT, 384], BF16, name="w_sb")
            proj0_sb = big.tile([64, C], BF16, name="proj0_sb")
            proj1_sb = big.tile([64, C], BF16, name="proj1_sb")
            qT_sb = big.tile([128, T], BF16, name="qT_sb")
            kT_sb = big.tile([128, T], BF16, name="kT_sb")
            vT_sb = big.tile([128, T], BF16, name="vT_sb")
            v_aug0 = big.tile([128, NK, 65], BF16, name="v_aug0")
            v_aug1 = big.tile([128, NK, 65], BF16, name="v_aug1")
            attn0 = big.tile([64, T], BF16, name="attn0")
            attn1 = big.tile([64, T], BF16, name="attn1")
            ident_sb = big.tile([128, 64], BF16, name="ident_sb")
            ones_sb = big.tile([128, 128], F32R, name="ones_sb")
            bias_sb = big.tile([128, 3], F32, name="bias_sb")

            # x and w tiles are fully-contiguous DRAM regions (full rows), so
            # each DMA streams at queue peak; pairs ordered so stage B's
            # matmuls start as soon as the first pair lands.
            # x tiles alternate between the two HWDGE queues (sync + scalar —
            # ScalarE is idle this early) so the load streams at 2x one
            # queue's ~100GB/s.
            for ct in range(NCT):
                nc.sync.dma_start(
                    out=w_sb[:, ct, :], in_=wqkv[128 * ct : 128 * ct + 128, :]
                )
                if ct == 0:
                    for q4 in range(4):
                        nc.scalar.dma_start(
                            out=x_sb[:, 0, CH * q4 : CH * q4 + CH],
                            in_=xT[0:128, CH * q4 : CH * q4 + CH],
                        )
                else:
                    nc.scalar.dma_start(
                        out=x_sb[:, ct, :], in_=xT[128 * ct : 128 * ct + 128, :]
                    )
            nc.sync.dma_start(out=bias_sb, in_=bias)
            nc.sync.dma_start(out=ident_sb, in_=identb)
            nc.sync.dma_start(out=ones_sb, in_=ones_f)
            nc.sync.dma_start(out=proj0_sb, in_=projT[0:64, :])
            nc.sync.dma_start(out=proj1_sb, in_=projT[64:128, :])

            nc.vector.memset(v_aug0[:, :, 64:65], 1.0)
            nc.vector.memset(v_aug1[:, :, 64:65], 1.0)

            # ---- stage B: q/k projections, two ct-outer sweeps -------------
            for sweep in range(2):
                grp = {}
                slots = [("s", 4), ("s", 4), ("s", 4), ("s", 4)]
                for part in (0, 1):
                    for c in (2 * sweep, 2 * sweep + 1):
                        tag, nbufs = slots.pop(0)
                        grp[(part, c)] = ps.tile(
                            [128, CH],
                            F32,
                            tag=tag,
                            bufs=nbufs,
                            name=f"qkps_{part}_{c}",
                        )
                for ct in range(NCT):
                    for (part, c), g in grp.items():
                        cols = slice(128 * part, 128 * part + 128)
                        nc.tensor.matmul(
                            g,
                            w_sb[:, ct, cols],
                            x_sb[:, ct, CH * c : CH * c + CH],
                            start=(ct == 0),
                            stop=(ct == NCT - 1),
                        )
                for (part, c), g in grp.items():
                    dest = qT_sb if part == 0 else kT_sb
                    nc.vector.tensor_scalar_add(
                        dest[:, CH * c : CH * c + CH], g, bias_sb[:, part : part + 1]
                    )

            # v projection + PE transposes, per chunk; chunk 0 up front, the
            # rest emitted as PE filler inside stage D's j-loops.
            def emit_v_chunk(c):
                v_ps = ps.tile([128, CH], F32, tag="m", bufs=2, name=f"vps_{c}")
                for ct in range(NCT):
                    nc.tensor.matmul(
                        v_ps,
                        w_sb[:, ct, 256:384],
                        x_sb[:, ct, CH * c : CH * c + CH],
                        start=(ct == 0),
                        stop=(ct == NCT - 1),
                    )
                nc.vector.tensor_scalar_add(
                    vT_sb[:, CH * c : CH * c + CH], v_ps, bias_sb[:, 2:3]
                )

            def transposes_for(c):
                for h, v_aug in ((0, v_aug0), (1, v_aug1)):
                    hrow = slice(64 * h, 64 * h + 64)
                    for kt in range(4 * c, 4 * c + 4):
                        tr_ps = ps.tile(
                            [128, 64], BF16, tag="m", bufs=2, name=f"tr_{h}_{kt}"
                        )
                        nc.tensor.transpose(
                            tr_ps,
                            vT_sb[hrow, 128 * kt : 128 * kt + 128],
                            ident_sb[hrow, :],
                        )
                        nc.vector.tensor_copy(v_aug[:, kt, 0:64], tr_ps)

            emit_v_chunk(0)
            transposes_for(0)
            emit_v_chunk(1)
            transposes_for(1)

            # ---- stages D+E per t-chunk ------------------------------------
            # Deferred work from chunk c-1, spread through chunk c's j-loop.
            pending_norm = None  # (at2, chunk)
            pending_proj = None  # chunk index

            def emit_norm(at2, pc):
                tcol = slice(CH * pc, CH * pc + CH)
                for h, attn in ((0, attn0), (1, attn1)):
                    rb_ps = ps.tile([128, CH], F32, tag="m", bufs=2, name=f"rb_{h}_{pc}")
                    nc.tensor.matmul(
                        rb_ps,
                        ones_sb[64:65, :],
                        at2[64:65, CH * h : CH * h + CH],
                        start=True,
                        stop=True,
                    )
                    nc.vector.tensor_mul(
                        attn[:, tcol], at2[0:64, CH * h : CH * h + CH], rb_ps[0:64, :]
                    )

            def emit_proj_tile(pc, m):
                tcol = slice(CH * pc, CH * pc + CH)
                pr_ps = ps.tile([128, CH], F32, tag="m", bufs=2, name=f"pr_{m}_{pc}")
                nc.tensor.matmul(
                    pr_ps,
                    proj0_sb[:, 128 * m : 128 * m + 128],
                    attn0[:, tcol],
                    start=True,
                    stop=False,
                )
                nc.tensor.matmul(
                    pr_ps,
                    proj1_sb[:, 128 * m : 128 * m + 128],
                    attn1[:, tcol],
                    start=False,
                    stop=True,
                )
                ob = outev_pool.tile([128, CH], BF16, tag="outev", name=f"ob_{m}_{pc}")
                if pc == 0 and m % 2:
                    nc.scalar.activation(
                        out=ob, in_=pr_ps, func=mybir.ActivationFunctionType.Copy
                    )
                else:
                    nc.vector.tensor_copy(ob, pr_ps)
                nc.sync.dma_start(out=out[pc, m], in_=ob)

            # Chunk order (1, 2, 3, 0): the ScalarE-heavy late chunks get the
            # previous chunk's projection matmuls as PE filler, and the final
            # chunk processed (0) has the shortest tail.
            for c in (1, 2, 3, 0):
                tcol = slice(CH * c, CH * c + CH)
                nj = 4 * c + 4
                pv_ps = {
                    h: ps.tile([65, CH], F32, tag="pv", bufs=2, name=f"pv_{h}_{c}")
                    for h in (0, 1)
                }
                pending = []
                proj_emitted = 0

                def emit_pv(step, last):
                    for (pj, ph, pw) in step:
                        nc.tensor.matmul(
                            pv_ps[ph],
                            (v_aug0 if ph == 0 else v_aug1)[:, pj, :],
                            pw,
                            start=(pj == 0),
                            stop=last,
                        )

                for j in range(nj):
                    for h in (0, 1):
                        hrow = slice(64 * h, 64 * h + 64)
                        s_ps = ps.tile(
                            [128, CH], F32, tag="s", bufs=4, name=f"s_{h}_{c}_{j}"
                        )
                        nc.tensor.matmul(
                            s_ps,
                            kT_sb[hrow, 128 * j : 128 * j + 128],
                            qT_sb[hrow, tcol],
                            start=True,
                            stop=True,
                        )
                        w_t = expw_pool.tile(
                            [128, CH], BF16, tag="expw", name=f"w_{h}_{c}_{j}"
                        )
                        diag = j - 4 * c
                        if diag <= 0:
                            nc.scalar.activation(out=w_t, in_=s_ps, func=EXP)
                        else:
                            # columns < 128*diag are fully masked: skip their
                            # exp, just zero them for the PV matmul read
                            lo = 128 * diag
                            nc.vector.memset(w_t[:, 0:lo], 0.0)
                            nc.scalar.activation(
                                out=w_t[:, lo:CH], in_=s_ps[:, lo:CH], func=EXP
                            )
                        if diag >= 0:
                            # keep exp(score) where t >= k: within the kept
                            # column range f' = f - 128*diag, so f' - p >= 0
                            lo = 128 * diag if diag > 0 else 0
                            nc.gpsimd.affine_select(
                                out=w_t[:, lo:CH],
                                in_=w_t[:, lo:CH],
                                pattern=[[1, CH - lo]],
                                compare_op=IS_GE,
                                fill=0.0,
                                base=0,
                                channel_multiplier=-1,
                            )
                        pending.append((j, h, w_t))
                    if j == 0 and pending_norm is not None:
                        emit_norm(*pending_norm)
                        pending_norm = None
                    if j == 1 and c == 1:
                        emit_v_chunk(2)
                    if j == 2 and c == 1:
                        transposes_for(2)
                    if j == 1 and c == 3:
                        emit_v_chunk(3)
                    if j == 3 and c == 3:
                        transposes_for(3)
                    while len(pending) > 2 * PIPE:
                        step, pending = pending[:2], pending[2:]
                        emit_pv(step, last=False)
                    if pending_proj is not None and j >= 1:
                        target = (j * 8) // max(nj - 1, 1)
                        while proj_emitted < min(target, 8):
                            emit_proj_tile(pending_proj, proj_emitted)
                            proj_emitted += 1
                while pending:
                    step, pending = pending[:2], pending[2:]
                    emit_pv(step, last=(len(pending) == 0))
                if pending_proj is not None:
                    while proj_emitted < 8:
                        emit_proj_tile(pending_proj, proj_emitted)
                        proj_emitted += 1

                at2 = attn_tmp_pool.tile(
                    [65, 2 * CH], F32R, tag="attn_tmp", name=f"at2_{c}"
                )
                for h in (0, 1):
                    nc.vector.tensor_copy(at2[:, CH * h : CH * h + CH], pv_ps[h])
                # 1/sums = exp(-ln(sums)); Ln+Exp share one act table set
                nc.scalar.activation(out=at2[64:65, :], in_=at2[64:65, :], func=LN)
                nc.scalar.activation(
                    out=at2[64:65, :], in_=at2[64:65, :], func=EXP, scale=-1.0
                )
                pending_norm = (at2, c)
                pending_proj = c

            emit_norm(*pending_norm)
            for m in range(8):
                emit_proj_tile(pending_proj, m)

    nc.compile()
    return nc


def _get_nc():
    if "nc" not in _CACHE:
        _CACHE["nc"] = _build()
    return _CACHE["nc"]


def _make_in_maps(x, wqkv_w, wqkv_b, proj_w):
    bf = ml_dtypes.bfloat16
    xT = np.ascontiguousarray(np.asarray(x, np.float32).T.astype(bf))
    identb = np.zeros((128, 64), bf)
    identb[0:64, :] = np.eye(64, dtype=bf)
    identb[64:128, :] = np.eye(64, dtype=bf)
    ones_f = np.ones((128, 128), np.float32)
    scale = np.float32(1.0 / np.sqrt(C))
    in_maps = []
    for i in range(N_CORES):
        rows = []
        biases = []
        for blk, s in ((0, scale), (1, None), (2, None)):
            sl = slice(blk * C + 128 * i, blk * C + 128 * i + 128)
            w = np.asarray(wqkv_w[sl], np.float32)
            b = np.asarray(wqkv_b[sl], np.float32)
            if s is not None:
                w = w * s
                b = b * s
            rows.append(w)
            biases.append(b)
        W = np.concatenate(rows, axis=0)  # [384, 1024]
        B = np.stack(biases, axis=1)  # [128, 3]
        pT = np.asarray(proj_w[:, 128 * i : 128 * i + 128], np.float32).T  # [128, 1024]
        in_maps.append(
            {
                "xT": xT,
                "wqkv": np.ascontiguousarray(W.T.astype(bf)),
                "projT": np.ascontiguousarray(pT.astype(bf)),
                "identb": identb,
                "ones_f": ones_f,
                "bias": np.ascontiguousarray(B),
            }
        )
    return in_maps


def kernel(x, wqkv_w, wqkv_b, proj_w, proj_b, _trace=False, _tmpdir=None):
    from concourse.bass_utils import run_bass_kernel_spmd

    nc = _get_nc()
    in_maps = _make_in_maps(x, wqkv_w, wqkv_b, proj_w)
    res = run_bass_kernel_spmd(
        nc,
        in_maps,
        core_ids=list(range(N_CORES)),
        trace=_trace,
        tmpdir=_tmpdir,
    )
    acc = np.zeros((NT, 8, 128, CH), np.float64)
    for rmap in res.results:
        acc += rmap["out"].astype(np.float64)
    partialT = acc.transpose(1, 2, 0, 3).reshape(C, T)  # [o, t]
    full = partialT.T + np.asarray(proj_b, np.float64)[None, :]
    if _trace:
        _CACHE["last_result"] = res
    return full.astype(np.float32)


# revision 42
# speedup vs baseline: 1.1241x; 1.1241x over previous
"""Causal self-attention (T=2048, C=1024, H=16) on 8 trn2 NeuronCores.

Tensor-parallel over heads: core i computes heads 2i, 2i+1 (q/k/v rows
128i:128i+128 of each 1024-row block of wqkv_w, proj_w columns
128i:128i+128), producing a partial output projection; partials are summed
on the host (the all-reduce of the sharding hint).

Per-core Bass/Tile kernel, bf16 matmuls with fp32 PSUM accumulation. The
PE instruction stream is kept dense (HAM re-throttles the PE clock to
1.2GHz after any ~3.4us idle/transpose-only window, and only re-warms
after ~3.4us of continuous matmul activity):
  B. qkvT[j, t] = wqkv.T @ xT, contraction-tile outer so matmuls chase the
     x DMAs; evacuation adds the (per-partition) bias and casts to bf16;
     q rows pre-scaled by 1/sqrt(C) on the host. v's PE transposes into
     v_aug[k, 65] (ones column = softmax denominator via the PV matmul)
     are interleaved with the v matmuls so they never form a
     transpose-only PE window.
  D. per 512-col t-chunk, heads interleaved, PV pipelined PIPE k-tile
     steps behind the scores, and the previous chunk's normalize/proj
     matmuls spread through the j-loop as PE filler while ScalarE
     (exp) catches up:
       sT[k, t] = kT.T @ qT -> exp (bf16, no max-subtraction needed:
       |scores| < ~1 by construction) -> causal affine_select on gpsimd
       (diagonal k-tiles only; fully-masked tiles skipped) ->
       attnT[d, t] += v_aug.T @ exp_w.
     Normalize: 1/sums via Ln+Exp(-x) on ScalarE (one activation table
     set holds both), partition-broadcast via K=1 float32r matmul with
     ones, one DVE multiply.
  E. partialT[o, t] = projT.T @ attn (two K=64 matmuls, one per head).
"""

import sys

if "/opt/trn_rl_repo" not in sys.path:
    sys.path.insert(0, "/opt/trn_rl_repo")

import ml_dtypes
import numpy as np

T = 2048
C = 1024
CH = 512  # t-chunk width (one PSUM bank of fp32)
NT = T // CH  # 4 t-chunks
NK = T // 128  # 16 k-tiles
NCT = C // 128  # 8 contraction tiles
N_CORES = 8
PIPE = 3  # scores->PV pipeline depth in k-tile steps

_CACHE = {}


def _patch_act_tables(bacc_mod, mybir):
    """Make Exp and Ln resolve to the one table set containing both, so the
    kernel needs a single ACT_TABLE_LOAD instead of thrashing between
    exp_and_others and natural_log_exp_and_others (~1.3us per reload)."""
    if getattr(bacc_mod, "_attn_act_patch", False):
        return
    orig = bacc_mod.get_activation_tables
    both = {mybir.ActivationFunctionType.Exp, mybir.ActivationFunctionType.Ln}

    def patched(arch):
        tabs = dict(orig(arch))
        return {
            name: (funcs if name == "natural_log_exp_and_others" else funcs - both)
            for name, funcs in tabs.items()
        }

    bacc_mod.get_activation_tables = patched
    bacc_mod._attn_act_patch = True


def _build():
    import concourse.tile as tile
    from concourse import bacc, mybir

    _patch_act_tables(bacc, mybir)

    F32 = mybir.dt.float32
    F32R = mybir.dt.float32r
    BF16 = mybir.dt.bfloat16
    EXP = mybir.ActivationFunctionType.Exp
    LN = mybir.ActivationFunctionType.Ln
    IS_GE = mybir.AluOpType.is_ge

    nc = bacc.Bacc(
        "TRN2",
        target_bir_lowering=False,
        debug=False,
        enable_asserts=False,
        num_devices=N_CORES,
        num_swdge_queues=4,
    )
    xT = nc.dram_tensor("xT", [C, T], BF16, kind="ExternalInput").ap()
    wqkv = nc.dram_tensor("wqkv", [C, 384], BF16, kind="ExternalInput").ap()
    projT = nc.dram_tensor("projT", [128, C], BF16, kind="ExternalInput").ap()
    identb = nc.dram_tensor("identb", [128, 64], BF16, kind="ExternalInput").ap()
    ones_f = nc.dram_tensor("ones_f", [128, 128], F32R, kind="ExternalInput").ap()
    bias = nc.dram_tensor("bias", [128, 3], F32, kind="ExternalInput").ap()
    # output as contiguous [chunk, o-tile, 128, 512] bf16 tiles: each store is
    # one fully-contiguous 128KB DMA (strided stores run at ~1/3 the queue
    # rate, and bf16 halves both the store bytes and the evacuation time)
    out = nc.dram_tensor("out", [NT, 8, 128, CH], BF16, kind="ExternalOutput").ap()

    with tile.TileContext(nc) as tc:
        with (
            tc.tile_pool(name="big", bufs=1) as big,
            tc.tile_pool(name="expw", bufs=8) as expw_pool,
            tc.tile_pool(name="attn_tmp", bufs=2) as attn_tmp_pool,
            tc.tile_pool(name="outev", bufs=3) as outev_pool,
            tc.tile_pool(name="ps", bufs=1, space="PSUM") as ps,
        ):
            # ---- resident SBUF tensors -------------------------------------
            x_sb = big.tile([128, NCT, T], BF16, name="x_sb")
            w_sb = big.tile([128, NCT, 384], BF16, name="w_sb")
            proj0_sb = big.tile([64, C], BF16, name="proj0_sb")
            proj1_sb = big.tile([64, C], BF16, name="proj1_sb")
            qT_sb = big.tile([128, T], BF16, name="qT_sb")
            kT_sb = big.tile([128, T], BF16, name="kT_sb")
            vT_sb = big.tile([128, T], BF16, name="vT_sb")
            v_aug0 = big.tile([128, NK, 65], BF16, name="v_aug0")
            v_aug1 = big.tile([128, NK, 65], BF16, name="v_aug1")
            attn0 = big.tile([64, T], BF16, name="attn0")
            attn1 = big.tile([64, T], BF16, name="attn1")
            ident_sb = big.tile([128, 64], BF16, name="ident_sb")
            ones_sb = big.tile([128, 128], F32R, name="ones_sb")
            bias_sb = big.tile([128, 3], F32, name="bias_sb")

            # x and w tiles are fully-contiguous DRAM regions (full rows), so
            # each DMA streams at queue peak; pairs ordered so stage B's
            # matmuls start as soon as the first pair lands.
            # x tiles alternate between the two HWDGE queues (sync + scalar —
            # ScalarE is idle this early) so the load streams at 2x one
            # queue's ~100GB/s.
            for ct in range(NCT):
                nc.sync.dma_start(
                    out=w_sb[:, ct, :], in_=wqkv[128 * ct : 128 * ct + 128, :]
                )
                if ct == 0:
                    for q4 in range(4):
                        nc.scalar.dma_start(
                            out=x_sb[:, 0, CH * q4 : CH * q4 + CH],
                            in_=xT[0:128, CH * q4 : CH * q4 + CH],
                        )
                else:
                    nc.scalar.dma_start(
                        out=x_sb[:, ct, :], in_=xT[128 * ct : 128 * ct + 128, :]
                    )
            nc.sync.dma_start(out=bias_sb, in_=bias)
            nc.sync.dma_start(out=ident_sb, in_=identb)
            nc.sync.dma_start(out=ones_sb, in_=ones_f)
            nc.sync.dma_start(out=proj0_sb, in_=projT[0:64, :])
            nc.sync.dma_start(out=proj1_sb, in_=projT[64:128, :])

            nc.vector.memset(v_aug0[:, :, 64:65], 1.0)
            nc.vector.memset(v_aug1[:, :, 64:65], 1.0)

            # ---- stage B: q/k projections, two ct-outer sweeps -------------
            for sweep in range(2):
                grp = {}
                slots = [("s", 4), ("s", 4), ("s", 4), ("s", 4)]
                for part in (0, 1):
                    for c in (2 * sweep, 2 * sweep + 1):
                        tag, nbufs = slots.pop(0)
                        grp[(part, c)] = ps.tile(
                            [128, CH],
                            F32,
                            tag=tag,
                            bufs=nbufs,
                            name=f"qkps_{part}_{c}",
                        )
                for ct in range(NCT):
                    for (part, c), g in grp.items():
                        cols = slice(128 * part, 128 * part + 128)
                        nc.tensor.matmul(
                            g,
                            w_sb[:, ct, cols],
                            x_sb[:, ct, CH * c : CH * c + CH],
                            start=(ct == 0),
                            stop=(ct == NCT - 1),
                        )
                for (part, c), g in grp.items():
                    dest = qT_sb if part == 0 else kT_sb
                    nc.vector.tensor_scalar_add(
                        dest[:, CH * c : CH * c + CH], g, bias_sb[:, part : part + 1]
                    )

            # v projection + PE transposes, per chunk; chunk 0 up front, the
            # rest emitted as PE filler inside stage D's j-loops.
            def emit_v_chunk(c):
                v_ps = ps.tile([128, CH], F32, tag="m", bufs=2, name=f"vps_{c}")
                for ct in range(NCT):
                    nc.tensor.matmul(
                        v_ps,
                        w_sb[:, ct, 256:384],
                        x_sb[:, ct, CH * c : CH * c + CH],
                        start=(ct == 0),
                        stop=(ct == NCT - 1),
                    )
                nc.vector.tensor_scalar_add(
                    vT_sb[:, CH * c : CH * c + CH], v_ps, bias_sb[:, 2:3]
                )

            def transposes_for(c):
                for h, v_aug in ((0, v_aug0), (1, v_aug1)):
                    hrow = slice(64 * h, 64 * h + 64)
                    for kt in range(4 * c, 4 * c + 4):
                        tr_ps = ps.tile(
                            [128, 64], BF16, tag="m", bufs=2, name=f"tr_{h}_{kt}"
                        )
                        nc.tensor.transpose(
                            tr_ps,
                            vT_sb[hrow, 128 * kt : 128 * kt + 128],
                            ident_sb[hrow, :],
                        )
                        nc.vector.tensor_copy(v_aug[:, kt, 0:64], tr_ps)

            emit_v_chunk(0)
            transposes_for(0)
            emit_v_chunk(1)
            transposes_for(1)

            # ---- stages D+E per t-chunk ------------------------------------
            # Deferred work from chunk c-1, spread through chunk c's j-loop.
            pending_norm = None  # (at2, chunk)
            pending_proj = None  # chunk index

            def emit_norm(at2, pc):
                tcol = slice(CH * pc, CH * pc + CH)
                for h, attn in ((0, attn0), (1, attn1)):
                    rb_ps = ps.tile([128, CH], F32, tag="m", bufs=2, name=f"rb_{h}_{pc}")
                    nc.tensor.matmul(
                        rb_ps,
                        ones_sb[64:65, :],
                        at2[64:65, CH * h : CH * h + CH],
                        start=True,
                        stop=True,
                    )
                    nc.vector.tensor_mul(
                        attn[:, tcol], at2[0:64, CH * h : CH * h + CH], rb_ps[0:64, :]
                    )

            def emit_proj_tile(pc, m):
                tcol = slice(CH * pc, CH * pc + CH)
                pr_ps = ps.tile([128, CH], F32, tag="m", bufs=2, name=f"pr_{m}_{pc}")
                nc.tensor.matmul(
                    pr_ps,
                    proj0_sb[:, 128 * m : 128 * m + 128],
                    attn0[:, tcol],
                    start=True,
                    stop=False,
                )
                nc.tensor.matmul(
                    pr_ps,
                    proj1_sb[:, 128 * m : 128 * m + 128],
                    attn1[:, tcol],
                    start=False,
                    stop=True,
                )
                ob = outev_pool.tile([128, CH], BF16, tag="outev", name=f"ob_{m}_{pc}")
                nc.vector.tensor_copy(ob, pr_ps)
                nc.sync.dma_start(out=out[pc, m], in_=ob)

            # Chunk order (1, 2, 3, 0): the ScalarE-heavy late chunks get the
            # previous chunk's projection matmuls as PE filler, and the final
            # chunk processed (0) has the shortest tail.
            for c in (1, 2, 3, 0):
                tcol = slice(CH * c, CH * c + CH)
                nj = 4 * c + 4
                pv_ps = {
                    h: ps.tile([65, CH], F32, tag="pv", bufs=2, name=f"pv_{h}_{c}")
                    for h in (0, 1)
                }
                pending = []
                proj_emitted = 0

                def emit_pv(step, last):
                    for (pj, ph, pw) in step:
                        nc.tensor.matmul(
                            pv_ps[ph],
                            (v_aug0 if ph == 0 else v_aug1)[:, pj, :],
                            pw,
                            start=(pj == 0),
                            stop=last,
                        )

                for j in range(nj):
                    for h in (0, 1):
                        hrow = slice(64 * h, 64 * h + 64)
                        s_ps = ps.tile(
                            [128, CH], F32, tag="s", bufs=4, name=f"s_{h}_{c}_{j}"
                        )
                        nc.tensor.matmul(
                            s_ps,
                            kT_sb[hrow, 128 * j : 128 * j + 128],
                            qT_sb[hrow, tcol],
                            start=True,
                            stop=True,
                        )
                        w_t = expw_pool.tile(
                            [128, CH], BF16, tag="expw", name=f"w_{h}_{c}_{j}"
                        )
                        diag = j - 4 * c
                        if diag <= 0:
                            nc.scalar.activation(out=w_t, in_=s_ps, func=EXP)
                        else:
                            # columns < 128*diag are fully masked: skip their
                            # exp, just zero them for the PV matmul read
                            lo = 128 * diag
                            nc.vector.memset(w_t[:, 0:lo], 0.0)
                            nc.scalar.activation(
                                out=w_t[:, lo:CH], in_=s_ps[:, lo:CH], func=EXP
                            )
                        if diag >= 0:
                            # keep exp(score) where t >= k: within the kept
                            # column range f' = f - 128*diag, so f' - p >= 0
                            lo = 128 * diag if diag > 0 else 0
                            nc.gpsimd.affine_select(
                                out=w_t[:, lo:CH],
                                in_=w_t[:, lo:CH],
                                pattern=[[1, CH - lo]],
                                compare_op=IS_GE,
                                fill=0.0,
                                base=0,
                                channel_multiplier=-1,
                            )
                        pending.append((j, h, w_t))
                    if j == 0 and pending_norm is not None:
                        emit_norm(*pending_norm)
                        pending_norm = None
                    if j == 1 and c == 1:
                        emit_v_chunk(2)
                    if j == 2 and c == 1:
                        transposes_for(2)
                    if j == 1 and c == 3:
                        emit_v_chunk(3)
                    if j == 3 and c == 3:
                        transposes_for(3)
                    while len(pending) > 2 * PIPE:
                        step, pending = pending[:2], pending[2:]
                        emit_pv(step, last=False)
                    if pending_proj is not None and j >= 1:
                        target = (j * 8) // max(nj - 1, 1)
                        while proj_emitted < min(target, 8):
                            emit_proj_tile(pending_proj, proj_emitted)
                            proj_emitted += 1
                while pending:
                    step, pending = pending[:2], pending[2:]
                    emit_pv(step, last=(len(pending) == 0))
                if pending_proj is not None:
                    while proj_emitted < 8:
                        emit_proj_tile(pending_proj, proj_emitted)
                        proj_emitted += 1

                at2 = attn_tmp_pool.tile(
                    [65, 2 * CH], F32R, tag="attn_tmp", name=f"at2_{c}"
                )
                for h in (0, 1):
                    nc.vector.tensor_copy(at2[:, CH * h : CH * h + CH], pv_ps[h])
                # 1/sums = exp(-ln(sums)); Ln+Exp share one act table set
                nc.scalar.activation(out=at2[64:65, :], in_=at2[64:65, :], func=LN)
                nc.scalar.activation(
                    out=at2[64:65, :], in_=at2[64:65, :], func=EXP, scale=-1.0
                )
                pending_norm = (at2, c)
                pending_proj = c

            emit_norm(*pending_norm)
            for m in range(8):
                emit_proj_tile(pending_proj, m)

    nc.compile()
    return nc


def _get_nc():
    if "nc" not in _CACHE:
        _CACHE["nc"] = _build()
    return _CACHE["nc"]


def _make_in_maps(x, wqkv_w, wqkv_b, proj_w):
    bf = ml_dtypes.bfloat16
    xT = np.ascontiguousarray(np.asarray(x, np.float32).T.astype(bf))
    identb = np.zeros((128, 64), bf)
    identb[0:64, :] = np.eye(64, dtype=bf)
    identb[64:128, :] = np.eye(64, dtype=bf)
    ones_f = np.ones((128, 128), np.float32)
    scale = np.float32(1.0 / np.sqrt(C))
    in_maps = []
    for i in range(N_CORES):
        rows = []
        biases = []
        for blk, s in ((0, scale), (1, None), (2, None)):
            sl = slice(blk * C + 128 * i, blk * C + 128 * i + 128)
            w = np.asarray(wqkv_w[sl], np.float32)
            b = np.asarray(wqkv_b[sl], np.float32)
            if s is not None:
                w = w * s
                b = b * s
            rows.append(w)
            biases.append(b)
        W = np.concatenate(rows, axis=0)  # [384, 1024]
        B = np.stack(biases, axis=1)  # [128, 3]
        pT = np.asarray(proj_w[:, 128 * i : 128 * i + 128], np.float32).T  # [128, 1024]
        in_maps.append(
            {
                "xT": xT,
                "wqkv": np.ascontiguousarray(W.T.astype(bf)),
                "projT": np.ascontiguousarray(pT.astype(bf)),
                "identb": identb,
                "ones_f": ones_f,
                "bias": np.ascontiguousarray(B),
            }
        )
    return in_maps


def kernel(x, wqkv_w, wqkv_b, proj_w, proj_b, _trace=False, _tmpdir=None):
    from concourse.bass_utils import run_bass_kernel_spmd

    nc = _get_nc()
    in_maps = _make_in_maps(x, wqkv_w, wqkv_b, proj_w)
    res = run_bass_kernel_spmd(
        nc,
        in_maps,
        core_ids=list(range(N_CORES)),
        trace=_trace,
        tmpdir=_tmpdir,
    )
    acc = np.zeros((NT, 8, 128, CH), np.float64)
    for rmap in res.results:
        acc += rmap["out"].astype(np.float64)
    partialT = acc.transpose(1, 2, 0, 3).reshape(C, T)  # [o, t]
    full = partialT.T + np.asarray(proj_b, np.float64)[None, :]
    if _trace:
        _CACHE["last_result"] = res
    return full.astype(np.float32)
